# revision 2
# baseline (speedup 1.0000x reference)
"""Trainium2 Bass kernel for nn_AttentionSHA (dense transformer attention block).

Full inputs -> full output. Internally: tensor-parallel over heads across 8
NeuronCores (core g owns kv-head g and query heads 4g..4g+3; wo row-sharded),
host-side reduce of the 8 partial output projections.

Math notes (validated against the reference in fp64/fp32 numpy):
  - The reference adds a 0/1 causal mask *before* softmax (no -inf masking) and
    runs softmax over the full MAXSEQ=2048 cache axis where positions >= S hold
    zero k/v. Softmax without max-subtraction is exact here (scores are in
    [-17, 18]), so:  out = sum_t exp(sc_t)*m_t*v_t / (sum_t exp(sc_t)*m_t + 1024)
    with m_t = e if visible else 1, and +1024 = (MAXSEQ - S) zero-score tail.
    The e-factor for fully-visible regions folds into the Exp bias
    (exp(x + 1) = e*exp(x)); only the 128x128 diagonal blocks need a mask mult.
  - RoPE is applied via host-permuted weight rows (even channels then odd), a
    partition-half swap, and two multiply-adds against [cos;cos] / [-sin;sin].
"""
import numpy as np
from contextlib import ExitStack

S = 1024
D = 4096
NH = 32
NKV = 8
HD = 128
NREP = NH // NKV          # 4
MAXSEQ = 2048
NCORES = 8
DT = D // 128             # 32 d-tiles
TT = S // 128             # 8 t-tiles

_CACHE = {}


def _build_nc():
    import concourse.bacc as bacc
    import concourse.mybir as mybir
    import concourse.tile as tile

    f32 = mybir.dt.float32
    f32r = mybir.dt.float32r
    Exp = mybir.ActivationFunctionType.Exp
    mult = mybir.AluOpType.mult
    add = mybir.AluOpType.add

    nc = bacc.Bacc("TRN2", target_bir_lowering=False, debug=False,
                   num_devices=NCORES)

    xT = nc.dram_tensor("xT", [D, S], f32, kind="ExternalInput")
    wq_t = nc.dram_tensor("wq_t", [NREP, D, HD], f32, kind="ExternalInput")
    wk_t = nc.dram_tensor("wk_t", [D, HD], f32, kind="ExternalInput")
    wv_t = nc.dram_tensor("wv_t", [D, HD], f32, kind="ExternalInput")
    wo_t = nc.dram_tensor("wo_t", [NREP * HD, D], f32, kind="ExternalInput")
    cc_d = nc.dram_tensor("cc", [HD, S], f32, kind="ExternalInput")
    ns_d = nc.dram_tensor("ns", [HD, S], f32, kind="ExternalInput")
    emaskd_d = nc.dram_tensor("emaskd", [TT, 128, 128], f32, kind="ExternalInput")
    ones_d = nc.dram_tensor("ones", [128, 128], f32, kind="ExternalInput")
    ident_d = nc.dram_tensor("ident", [128, 128], f32, kind="ExternalInput")
    outT = nc.dram_tensor("outT", [D, S], f32, kind="ExternalOutput")

    with tile.TileContext(nc) as tc, ExitStack() as ctx:
        const = ctx.enter_context(tc.tile_pool(name="const", bufs=1))
        wts = ctx.enter_context(tc.tile_pool(name="wts", bufs=6))
        xpool = ctx.enter_context(tc.tile_pool(name="xpool", bufs=3))
        rpool = ctx.enter_context(tc.tile_pool(name="rpool", bufs=2))
        qkv = ctx.enter_context(tc.tile_pool(name="qkv", bufs=1))
        hs = ctx.enter_context(tc.tile_pool(name="hs", bufs=5))
        epool = ctx.enter_context(tc.tile_pool(name="epool", bufs=4))
        zpool = ctx.enter_context(tc.tile_pool(name="zpool", bufs=1))
        opool = ctx.enter_context(tc.tile_pool(name="opool", bufs=2))
        ps = ctx.enter_context(tc.tile_pool(name="ps", bufs=8, space="PSUM"))

        # ---- constants ----
        cc_sb = const.tile([128, S], f32)
        ns_sb = const.tile([128, S], f32)
        ones_sb = const.tile([128, 128], f32r)
        ident_sb = const.tile([128, 128], f32)
        nc.sync.dma_start(cc_sb[:], cc_d[:])
        nc.sync.dma_start(ns_sb[:], ns_d[:])
        nc.gpsimd.dma_start(ones_sb[:], ones_d[:])
        nc.sync.dma_start(ident_sb[:], ident_d[:])

        emaskd_sb = const.tile([128, TT * 128], f32)
        nc.sync.dma_start(
            emaskd_sb[:].rearrange("p (t e) -> p t e", t=TT),
            emaskd_d[:].rearrange("t p e -> p t e"),
        )

        # ---- weights (f32r via casting DMA); wo reuses these slots later ----
        wq_sb = []
        for h in range(NREP):
            w = wts.tile([128, D], f32r, name=f"wq_sb{h}", tag="w16")
            nc.gpsimd.dma_start(
                w[:].rearrange("p (t e) -> p t e", t=DT),
                wq_t[h].rearrange("(t p) e -> p t e", p=128),
            )
            wq_sb.append(w)
        wk_sb = wts.tile([128, D], f32r, tag="w16")
        nc.gpsimd.dma_start(
            wk_sb[:].rearrange("p (t e) -> p t e", t=DT),
            wk_t[:].rearrange("(t p) e -> p t e", p=128),
        )
        wv_sb = wts.tile([128, D], f32r, tag="w16")
        nc.gpsimd.dma_start(
            wv_sb[:].rearrange("p (t e) -> p t e", t=DT),
            wv_t[:].rearrange("(t p) e -> p t e", p=128),
        )

        # ---- phase 1: QKV projections + RoPE ----
        q_rot = [hs.tile([128, S], f32r, name=f"q_rot{h}", tag="hs")
                 for h in range(NREP)]                      # per head [e, s]
        k_rot = qkv.tile([128, S], f32r)                    # [e, t]
        v_et = qkv.tile([128, S], f32)                      # [e, t] pre-transpose

        for sh in range(2):
            s0 = 512 * sh
            q_ps = [ps.tile([128, 512], f32, tag="ps", name=f"q_ps{sh}_{h}")
                    for h in range(NREP)]
            k_ps = ps.tile([128, 512], f32, tag="ps", name=f"k_ps{sh}")
            v_ps = ps.tile([128, 512], f32, tag="ps", name=f"v_ps{sh}")
            for d in range(DT):
                x_r = xpool.tile([128, 512], f32r, name="x_r")
                nc.gpsimd.dma_start(x_r[:], xT[128 * d:128 * (d + 1), s0:s0 + 512])
                for h in range(NREP):
                    nc.tensor.matmul(q_ps[h][:], wq_sb[h][:, 128 * d:128 * (d + 1)],
                                     x_r[:], start=(d == 0), stop=(d == DT - 1))
                nc.tensor.matmul(k_ps[:], wk_sb[:, 128 * d:128 * (d + 1)],
                                 x_r[:], start=(d == 0), stop=(d == DT - 1))
                nc.tensor.matmul(v_ps[:], wv_sb[:, 128 * d:128 * (d + 1)],
                                 x_r[:], start=(d == 0), stop=(d == DT - 1))

            # RoPE: dest = psum*[cos;cos] + swap(psum)*[-sin;sin]
            def rope(psum, dest):
                sw = rpool.tile([128, 512], f32, name="sw")
                nc.scalar.copy(sw[0:64, :], psum[64:128, :])
                nc.scalar.copy(sw[64:128, :], psum[0:64, :])
                t1 = rpool.tile([128, 512], f32, name="t1")
                nc.vector.tensor_tensor(t1[:], psum[:], cc_sb[:, s0:s0 + 512], op=mult)
                t2 = rpool.tile([128, 512], f32, name="t2")
                nc.vector.tensor_tensor(t2[:], sw[:], ns_sb[:, s0:s0 + 512], op=mult)
                nc.vector.tensor_tensor(dest, t1[:], t2[:], op=add)

            for h in range(NREP):
                rope(q_ps[h], q_rot[h][:, s0:s0 + 512])
            rope(k_ps, k_rot[:, s0:s0 + 512])
            nc.vector.tensor_copy(v_et[:, s0:s0 + 512], v_ps[:])

        # ---- phase 2: transpose V to [t, e] ----
        v_te = qkv.tile([128, TT * 128], f32r)
        for t in range(TT):
            tr = ps.tile([128, 128], f32, tag="ps", name="tr")
            nc.tensor.transpose(tr[:], v_et[:, 128 * t:128 * (t + 1)], ident_sb[:])
            nc.vector.tensor_copy(v_te[:, 128 * t:128 * (t + 1)], tr[:])

        # ---- phase 3: attention per head ----
        att = []                                  # per head [e, s], normalized
        inv_sqrt_hd = float(1.0 / np.sqrt(HD))
        for h in range(NREP):
            z_ps = [ps.tile([128, 512], f32, tag="ps", name=f"z_ps{h}_{c}")
                    for c in range(2)]
            o_ps = [ps.tile([128, 512], f32, tag="ps", name=f"o_ps{h}_{c}")
                    for c in range(2)]
            for t in range(TT):
                dlo, dhi = 128 * t, 128 * (t + 1)
                expm = epool.tile([128, S], f32r, name="expm")
                for c in range(2):
                    sc = ps.tile([128, 512], f32, tag="ps", name="sc")
                    nc.tensor.matmul(sc[:], k_rot[:, dlo:dhi],
                                     q_rot[h][:, 512 * c:512 * (c + 1)],
                                     start=True, stop=True)
                    lo, hi = 512 * c, 512 * (c + 1)
                    if dlo >= hi:
                        # fully invisible: plain exp
                        nc.scalar.activation(expm[:, lo:hi], sc[:], Exp,
                                             scale=inv_sqrt_hd)
                    elif dhi <= lo:
                        # fully visible: exp(x + 1) = e * exp(x)
                        nc.scalar.activation(expm[:, lo:hi], sc[:], Exp,
                                             scale=inv_sqrt_hd, bias=1.0)
                    else:
                        # diagonal block inside this chunk
                        if dlo > lo:
                            nc.scalar.activation(expm[:, lo:dlo],
                                                 sc[:, 0:dlo - lo], Exp,
                                                 scale=inv_sqrt_hd)
                        et = epool.tile([128, 128], f32, name="et")
                        nc.scalar.activation(et[:], sc[:, dlo - lo:dhi - lo],
                                             Exp, scale=inv_sqrt_hd)
                        nc.vector.tensor_tensor(
                            expm[:, dlo:dhi], et[:],
                            emaskd_sb[:, 128 * t:128 * (t + 1)], op=mult)
                        if dhi < hi:
                            nc.scalar.activation(expm[:, dhi:hi],
                                                 sc[:, dhi - lo:512], Exp,
                                                 scale=inv_sqrt_hd, bias=1.0)
                for c in range(2):
                    nc.tensor.matmul(z_ps[c][:], ones_sb[:],
                                     expm[:, 512 * c:512 * (c + 1)],
                                     start=(t == 0), stop=(t == TT - 1))
                    nc.tensor.matmul(o_ps[c][:], v_te[:, 128 * t:128 * (t + 1)],
                                     expm[:, 512 * c:512 * (c + 1)],
                                     start=(t == 0), stop=(t == TT - 1))
            z_sb = zpool.tile([128, S], f32, name="z_sb")
            rz = zpool.tile([128, S], f32, name="rz")
            for c in range(2):
                nc.vector.tensor_scalar_add(z_sb[:, 512 * c:512 * (c + 1)],
                                            z_ps[c][:], float(MAXSEQ - S))
            nc.vector.reciprocal(rz[:], z_sb[:])
            a = hs.tile([128, S], f32r, name=f"att{h}", tag="hs")
            for c in range(2):
                nc.vector.tensor_tensor(a[:, 512 * c:512 * (c + 1)],
                                        o_ps[c][:], rz[:, 512 * c:512 * (c + 1)],
                                        op=mult)
            att.append(a)

        # ---- phase 4: output projection (partial over this core's 512 cols) ----
        wo_sb = []
        for h in range(NREP):
            w = wts.tile([128, D], f32r, name=f"wo_sb{h}", tag="w16")
            nc.gpsimd.dma_start(w[:], wo_t[128 * h:128 * (h + 1), :])
            wo_sb.append(w)

        for do in range(DT):
            op_ps = [ps.tile([128, 512], f32, tag="ps", name=f"op{c}")
                     for c in range(2)]
            for c in range(2):
                for h in range(NREP):
                    nc.tensor.matmul(op_ps[c][:],
                                     wo_sb[h][:, 128 * do:128 * (do + 1)],
                                     att[h][:, 512 * c:512 * (c + 1)],
                                     start=(h == 0), stop=(h == NREP - 1))
            out_sb = opool.tile([128, S], f32, name="out_sb")
            for c in range(2):
                nc.vector.tensor_copy(out_sb[:, 512 * c:512 * (c + 1)],
                                      op_ps[c][:])
            nc.sync.dma_start(outT[128 * do:128 * (do + 1), :], out_sb[:])

    nc.compile()
    return nc


def kernel(**inputs):
    from concourse.bass_utils import run_bass_kernel_spmd

    x = np.asarray(inputs["x"], np.float32)                 # [1, S, D]
    cos = np.asarray(inputs["freqs_cos"], np.float32)       # [S, 64]
    sin = np.asarray(inputs["freqs_sin"], np.float32)       # [S, 64]
    wq = np.asarray(inputs["wq"], np.float32)               # [NH, HD, D]
    wk = np.asarray(inputs["wk"], np.float32)               # [NKV, HD, D]
    wv = np.asarray(inputs["wv"], np.float32)               # [NKV, HD, D]
    wo = np.asarray(inputs["wo"], np.float32)               # [D, D]
    input_pos = np.asarray(inputs["input_pos"]).astype(np.int64)  # [S]

    if "nc" not in _CACHE:
        _CACHE["nc"] = _build_nc()
    nc = _CACHE["nc"]

    perm = np.concatenate([np.arange(0, HD, 2), np.arange(1, HD, 2)])
    xT = np.ascontiguousarray(x[0].T)                       # [D, S]
    cc = np.ascontiguousarray(np.concatenate([cos.T, cos.T], 0))   # [128, S]
    ns = np.ascontiguousarray(np.concatenate([-sin.T, sin.T], 0))  # [128, S]
    # visibility adds +1 pre-exp where input_pos[t] <= input_pos[s]; for the
    # (spec-guaranteed) sorted arange fill only diagonal blocks are mixed.
    emaskd = np.empty((TT, 128, 128), np.float32)
    for t in range(TT):
        p = input_pos[128 * t:128 * (t + 1)]
        emaskd[t] = np.where(p[:, None] <= p[None, :], np.float32(np.e),
                             np.float32(1.0))
    ones128 = np.ones((128, 128), np.float32)
    ident = np.eye(128, dtype=np.float32)

    in_maps = []
    for g in range(NCORES):
        wq_g = wq[NREP * g:NREP * (g + 1)][:, perm, :]       # [4, 128, D]
        in_maps.append({
            "xT": xT,
            "wq_t": np.ascontiguousarray(wq_g.transpose(0, 2, 1)),   # [4, D, 128]
            "wk_t": np.ascontiguousarray(wk[g][perm].T),             # [D, 128]
            "wv_t": np.ascontiguousarray(wv[g].T),                   # [D, 128]
            "wo_t": np.ascontiguousarray(
                wo[:, NREP * HD * g:NREP * HD * (g + 1)].T),         # [512, D]
            "cc": cc, "ns": ns, "emaskd": emaskd,
            "ones": ones128, "ident": ident,
        })

    res = run_bass_kernel_spmd(nc, in_maps, list(range(NCORES)))
    total = np.zeros((D, S), np.float64)
    for g in range(NCORES):
        total += res.results[g]["outT"]
    return np.ascontiguousarray(total.T.astype(np.float32)[None])   # [1, S, D]
